# revision 7
# baseline (speedup 1.0000x reference)
"""Causal GQA multi-head attention on 8 TRN2 NeuronCores.

Sharding: data-parallel over batch (B=8 -> one batch element per core,
weights replicated, no collectives).

Per-core kernel (T=1024, C=576, 9 q-heads / 3 kv-heads, hd=64):
  - x [T, C] f32 is loaded and transposed on-chip (PE transpose) into
    xT [C(+1 ones row), T] bf16.
  - qT = (Wq|bq)^T-style projections: the bias is folded into the matmul
    by appending a ones row to xT and a bias row to each weight matrix.
    qT/kT are produced channel-major (what attention needs); v is produced
    token-major with a ones column appended (denominator trick).
  - scores are computed transposed, S^T[tk, tq] = (k_tile)^T-block @ qT,
    exp() on ScalarE (scale 1/sqrt(hd) folded into q), causal handled by
    only computing lower blocks + a 0/1 upper-triangular mask multiply on
    the diagonal blocks.
  - y[tq, d] (+ row-sum column l[tq]) accumulates P~^T-block.T @ [v|1] in
    PSUM; normalize with reciprocal + per-partition tensor_scalar.
  - y is PE-transposed to yT (+ones row) and projected by (Wo|bo).
"""

import sys

for _p in ("/opt/trn_rl_repo",):
    if _p not in sys.path:
        sys.path.insert(0, _p)

from contextlib import ExitStack

import ml_dtypes
import numpy as np

import concourse.bass as bass
import concourse.mybir as mybir
import concourse.tile as tile
from concourse import bacc
from concourse.bass_utils import run_bass_kernel_spmd
from concourse.masks import make_identity, make_upper_triangular

B, T, C = 8, 1024, 576
NH, NKV, HD = 9, 3, 64
KVC = C // NKV * NKV // 3  # 192
KVC = 192
NREP = NH // NKV  # 3
NKT = 5  # channel k-tiles: 4 x 128 + 64(+1 ones row)
NTT = T // 128  # 8 token tiles
F32 = mybir.dt.float32
BF16 = mybir.dt.bfloat16
SCALE = 1.0 / float(np.sqrt(HD))

N_CORES = 8


def _cw(ki):
    """channel-tile row count (without ones row)"""
    return 128 if ki < NKT - 1 else C - 128 * (NKT - 1)  # 64


def _kw(ki):
    """channel-tile row count as matmul K (incl. ones row on last tile)"""
    return 128 if ki < NKT - 1 else C - 128 * (NKT - 1) + 1  # 65


def build_kernel(tc, ctx, x, wq, wk, wv, wo, out):
    nc = tc.nc

    consts = ctx.enter_context(tc.tile_pool(name="consts", bufs=1))
    persist = ctx.enter_context(tc.tile_pool(name="persist", bufs=1))

    # --- constants: weights, identities, diag mask -------------------------
    ident_f32 = consts.tile([128, 128], F32, tag="idf")
    make_identity(nc, ident_f32)
    ident_bf16 = consts.tile([128, 128], BF16, tag="idb")
    make_identity(nc, ident_bf16)
    # 1 on/above diagonal, 0 below: multiplies exp(S^T) diagonal blocks
    # (keep tk <= tq).
    m01 = consts.tile([128, 128], BF16, tag="m01")
    make_upper_triangular(nc, m01, val=1.0, diag=True)

    wq_sb, wk_sb, wv_sb, wo_sb = [], [], [], []
    for ki in range(NKT):
        kw = _kw(ki)
        r0 = 128 * ki
        wq_t = consts.tile([kw, C], BF16, tag=f"wq{ki}")
        nc.sync.dma_start(out=wq_t, in_=wq[r0 : r0 + kw, :])
        wq_sb.append(wq_t)
        wk_t = consts.tile([kw, KVC], BF16, tag=f"wk{ki}")
        nc.sync.dma_start(out=wk_t, in_=wk[r0 : r0 + kw, :])
        wk_sb.append(wk_t)
        wv_t = consts.tile([kw, KVC], BF16, tag=f"wv{ki}")
        nc.sync.dma_start(out=wv_t, in_=wv[r0 : r0 + kw, :])
        wv_sb.append(wv_t)
        wo_t = consts.tile([kw, C], BF16, tag=f"wo{ki}")
        nc.sync.dma_start(out=wo_t, in_=wo[r0 : r0 + kw, :])
        wo_sb.append(wo_t)

    # --- persistent activations -------------------------------------------
    xT = []  # channel-major x, last tile has ones row at row 64
    yT = []  # channel-major attention out, ones row likewise
    for ki in range(NKT):
        kw = _kw(ki)
        xT_t = persist.tile([kw, T], BF16, tag=f"xT{ki}")
        xT.append(xT_t)
        yT_t = persist.tile([kw, T], BF16, tag=f"yT{ki}")
        yT.append(yT_t)
    nc.vector.memset(xT[NKT - 1][_cw(NKT - 1) : _kw(NKT - 1), :], 1.0)
    nc.vector.memset(yT[NKT - 1][_cw(NKT - 1) : _kw(NKT - 1), :], 1.0)

    # per-head channel-major q (scaled by 1/sqrt(hd)) and per-kv-head k,
    # each at base partition 0 (matmul requires lhsT/rhs base match)
    qT = []
    for h in range(NH):
        qT_t = persist.tile([HD, T], BF16, tag=f"qT{h}")
        qT.append(qT_t)
    kT = []
    for g in range(NKV):
        kT_t = persist.tile([HD, T], BF16, tag=f"kT{g}")
        kT.append(kT_t)

    v_aug = []  # per token tile: [128, NKV, 65]; col 64 = ones
    for tt in range(NTT):
        v_t = persist.tile([128, NKV, HD + 1], BF16, tag=f"v{tt}")
        nc.vector.memset(v_t[:, :, HD : HD + 1], 1.0)
        v_aug.append(v_t)

    y_sb = []  # token-major normalized attention out, bf16
    for tt in range(NTT):
        y_t = persist.tile([128, C], BF16, tag=f"y{tt}")
        y_sb.append(y_t)

    # --- phase 1: load x, transpose to xT ---------------------------------
    with (
        tc.tile_pool(name="xload", bufs=3) as xload,
        tc.tile_pool(name="tps", bufs=4, space="PSUM") as tps,
    ):
        for tt in range(NTT):
            xt = xload.tile([128, C], F32, tag="xt")
            nc.sync.dma_start(out=xt, in_=x[128 * tt : 128 * (tt + 1), :])
            for ki in range(NKT):
                cw = _cw(ki)
                ps = tps.tile([128, 128], F32, tag="tp")
                nc.tensor.transpose(
                    ps[:cw, :], xt[:, 128 * ki : 128 * ki + cw], ident_f32
                )
                nc.vector.tensor_copy(
                    xT[ki][0:cw, 128 * tt : 128 * (tt + 1)], ps[:cw, :]
                )

    # --- phase 2: q/k/v projections ---------------------------------------
    with tc.tile_pool(name="qkv_ps", bufs=2, space="PSUM") as qkv_ps:
        # qT[c_out, t] = sum_c wq[c, c_out] * xT[c, t]  (+bias via ones row)
        for h in range(NH):
            for ni in range(2):
                n0 = 512 * ni
                ps = qkv_ps.tile([HD, 512], F32, tag="proj")
                for ki in range(NKT):
                    nc.tensor.matmul(
                        ps,
                        lhsT=wq_sb[ki][:, HD * h : HD * (h + 1)],
                        rhs=xT[ki][:, n0 : n0 + 512],
                        start=(ki == 0),
                        stop=(ki == NKT - 1),
                    )
                # scale by 1/sqrt(hd) while evacuating (cast to bf16)
                nc.vector.tensor_scalar_mul(qT[h][:, n0 : n0 + 512], ps, SCALE)
        # kT[c_out, t]
        for g in range(NKV):
            for ni in range(2):
                n0 = 512 * ni
                ps = qkv_ps.tile([HD, 512], F32, tag="proj")
                for ki in range(NKT):
                    nc.tensor.matmul(
                        ps,
                        lhsT=wk_sb[ki][:, HD * g : HD * (g + 1)],
                        rhs=xT[ki][:, n0 : n0 + 512],
                        start=(ki == 0),
                        stop=(ki == NKT - 1),
                    )
                nc.vector.tensor_copy(kT[g][:, n0 : n0 + 512], ps)
        # v[t, c'] token-major
        for tt in range(NTT):
            ps = qkv_ps.tile([128, KVC], F32, tag="vproj")
            for ki in range(NKT):
                nc.tensor.matmul(
                    ps,
                    lhsT=xT[ki][:, 128 * tt : 128 * (tt + 1)],
                    rhs=wv_sb[ki],
                    start=(ki == 0),
                    stop=(ki == NKT - 1),
                )
            nc.vector.tensor_copy(
                v_aug[tt][:, :, 0:HD],
                ps.rearrange("p (g d) -> p g d", g=NKV),
            )

    # --- phase 3: attention, one kv-group (3 q-heads) at a time -----------
    for g in range(NKV):
        with (
            tc.tile_pool(name=f"pexp{g}", bufs=1) as pexp,
            tc.tile_pool(name=f"sc{g}", bufs=3, space="PSUM") as sc_ps,
            tc.tile_pool(name=f"pv{g}", bufs=2, space="PSUM") as pv_ps,
            tc.tile_pool(name=f"z{g}", bufs=2) as zpool,
        ):
            p_sb = {}
            for hl in range(NREP):
                h = NREP * g + hl
                for j in range(NTT):
                    nq = T - 128 * j  # causal: tq >= 128*j
                    ps = sc_ps.tile([128, 1024], F32, tag="s")
                    for c0 in range(0, nq, 512):
                        cn = min(512, nq - c0)
                        nc.tensor.matmul(
                            ps[:, c0 : c0 + cn],
                            lhsT=kT[g][:, 128 * j : 128 * (j + 1)],
                            rhs=qT[h][:, 128 * j + c0 : 128 * j + c0 + cn],
                            start=True,
                            stop=True,
                        )
                    pt = pexp.tile([128, 1024], BF16, tag=f"p{hl}_{j}")
                    nc.scalar.activation(
                        pt[:, 0:nq], ps[:, 0:nq], mybir.ActivationFunctionType.Exp
                    )
                    # causal mask inside the diagonal block: zero tk > tq
                    nc.vector.tensor_mul(pt[:, 0:128], pt[:, 0:128], m01)
                    p_sb[(hl, j)] = pt

            for i in range(NTT):
                yps = pv_ps.tile([128, NREP, HD + 1], F32, tag="y")
                for hl in range(NREP):
                    for j in range(i + 1):
                        nc.tensor.matmul(
                            yps[:, hl, :],
                            lhsT=p_sb[(hl, j)][:, 128 * (i - j) : 128 * (i - j) + 128],
                            rhs=v_aug[j][:, g, :],
                            start=(j == 0),
                            stop=(j == i),
                        )
                z3 = zpool.tile([128, NREP], F32, tag="z")
                nc.vector.reciprocal(z3, yps[:, :, HD])
                for hl in range(NREP):
                    h = NREP * g + hl
                    nc.vector.tensor_scalar_mul(
                        y_sb[i][:, HD * h : HD * (h + 1)],
                        yps[:, hl, 0:HD],
                        z3[:, hl : hl + 1],
                    )

    # --- phase 4: transpose y -> yT ---------------------------------------
    with tc.tile_pool(name="ytp", bufs=4, space="PSUM") as ytp:
        for tt in range(NTT):
            for ki in range(NKT):
                cw = _cw(ki)
                ps = ytp.tile([128, 128], BF16, tag="yt")
                nc.tensor.transpose(
                    ps[:cw, :], y_sb[tt][:, 128 * ki : 128 * ki + cw], ident_bf16
                )
                nc.vector.tensor_copy(
                    yT[ki][0:cw, 128 * tt : 128 * (tt + 1)], ps[:cw, :]
                )

    # --- phase 5: output projection ---------------------------------------
    with (
        tc.tile_pool(name="ops", bufs=2, space="PSUM") as ops,
        tc.tile_pool(name="osb", bufs=3) as osb,
    ):
        for tt in range(NTT):
            ps = ops.tile([128, 1024], F32, tag="o")
            for c0, cn in ((0, 512), (512, 64)):
                for ki in range(NKT):
                    nc.tensor.matmul(
                        ps[:, c0 : c0 + cn],
                        lhsT=yT[ki][:, 128 * tt : 128 * (tt + 1)],
                        rhs=wo_sb[ki][:, c0 : c0 + cn],
                        start=(ki == 0),
                        stop=(ki == NKT - 1),
                    )
            o_sb = osb.tile([128, C], F32, tag="ot")
            nc.vector.tensor_copy(o_sb, ps[:, 0:C])
            nc.sync.dma_start(out=out[128 * tt : 128 * (tt + 1), :], in_=o_sb)


def build_bass():
    # Bacc (not raw Bass): its finalize() runs move_matmul_waits_to_ldweights
    # + generate_event_semaphores, required to satisfy the 1-wait-per-
    # instruction hardware constraint that walrus enforces.
    nc = bacc.Bacc("TRN2", target_bir_lowering=False)
    x = nc.declare_dram_parameter("x", [T, C], F32, isOutput=False)
    wq = nc.declare_dram_parameter("wq", [C + 1, C], BF16, isOutput=False)
    wk = nc.declare_dram_parameter("wk", [C + 1, KVC], BF16, isOutput=False)
    wv = nc.declare_dram_parameter("wv", [C + 1, KVC], BF16, isOutput=False)
    wo = nc.declare_dram_parameter("wo", [C + 1, C], BF16, isOutput=False)
    out = nc.declare_dram_parameter("out", [T, C], F32, isOutput=True)
    with tile.TileContext(nc) as tc, ExitStack() as ctx:
        build_kernel(tc, ctx, x[:], wq[:], wk[:], wv[:], wo[:], out[:])
    nc.finalize()  # runs Bacc.compile(): reg alloc + wait splitting
    return nc


_NC_CACHE = None


def _get_nc():
    global _NC_CACHE
    if _NC_CACHE is None:
        _NC_CACHE = build_bass()
    return _NC_CACHE


def prep_inputs(x, Wq, bq, Wk, bk, Wv, bv, Wo, bo):
    """Host-side: fold biases into an extra weight row, cast weights bf16."""
    bf = ml_dtypes.bfloat16
    wq = np.concatenate([Wq, bq[None, :]], axis=0).astype(bf)
    wk = np.concatenate([Wk, bk[None, :]], axis=0).astype(bf)
    wv = np.concatenate([Wv, bv[None, :]], axis=0).astype(bf)
    wo = np.concatenate([Wo, bo[None, :]], axis=0).astype(bf)
    x = np.ascontiguousarray(np.asarray(x, dtype=np.float32))
    in_maps = [
        {"x": x[b], "wq": wq, "wk": wk, "wv": wv, "wo": wo} for b in range(N_CORES)
    ]
    return in_maps


def kernel(x, Wq, bq, Wk, bk, Wv, bv, Wo, bo, _trace=False, _trace_kwargs=None):
    nc = _get_nc()
    in_maps = prep_inputs(x, Wq, bq, Wk, bk, Wv, bv, Wo, bo)
    res = run_bass_kernel_spmd(
        nc,
        in_maps,
        core_ids=list(range(N_CORES)),
        trace=_trace,
        **(_trace_kwargs or {}),
    )
    out = np.stack([res.results[b]["out"] for b in range(N_CORES)], axis=0)
    if _trace:
        return out.astype(np.float32), res
    return out.astype(np.float32)
